# revision 1
# baseline (speedup 1.0000x reference)
"""Trainium2 Bass kernel for nn_Attention_13073880449373.

Full-batch multi-head attention (B=8, S=1024, C=1024, H=16, D=64) with RoPE,
data-parallel over the batch dim: core b computes batch b end-to-end.

Per-core dataflow (all "T" = channels-on-partitions layout):
  xT (C,S)  --[W_qk as stationary]-->  qkT (2C, S) + per-partition bias (ACT)
  xT (C,S)  --[xT as stationary]  -->  v   (S, C) + bias row via K=1 matmul,
                                       staged to DRAM with interleaved ones col
  RoPE on qkT (DVE; SBUF->SBUF DMA for the rotate-half partition swap)
  scoresT (Sk,Sq) = k'T.T @ q'T per head (K=64)
  pT = exp(0.125 * scoresT)            (ACT, PSUM->SBUF, fp32r out)
  outT (65, Sq) = [v|1].T @ pT         (row 64 = softmax denominators)
  recip = 1/outT[64] (DVE), broadcast over partitions (GPSIMD),
  normalize (DVE) -> attn_outT staged to DRAM
  out (S, C) = attn_outT.T @ W_proj + bias row (K=1 matmul)
All matmul operands live in float32r tiles (full-rate fp32 mode at N=512).
The qk->attention pipeline is interleaved per head-pair to keep PE dense.
"""

import math
import os
from contextlib import ExitStack

import numpy as np

B, S, C = 8, 1024, 1024
H, D = 16, 64
N_CORES = 8
KC = C // 128  # 8 contraction chunks of 128

_CACHE = {}


def _cs_table():
    # Matches reference.rope_cos_sin computed in float32, transposed, with the
    # rotate-half sign folded into the sin half (rows 0-31 negated).
    f = np.float32
    inv = np.exp(np.arange(0, D, 2, dtype=f) * f(-(math.log(10000.0) / D))).astype(f)
    pos = np.arange(S, dtype=f)[:, None]
    ang = (pos * inv[None, :]).astype(f)  # (S, 32)
    ang = np.concatenate([ang, ang], axis=1)  # (S, 64)
    cosT = np.cos(ang).T.astype(f)  # (64, S)
    sinT = np.sin(ang).T.astype(f)
    sign = np.where(np.arange(D) < D // 2, f(-1.0), f(1.0))[:, None].astype(f)
    half = np.concatenate([cosT, sinT * sign], axis=1)  # (64, 2S)
    return np.concatenate([half, half], axis=0).astype(f)  # (128, 2S)


def declare_io(nc):
    from concourse import mybir

    f32 = mybir.dt.float32
    return {
        "xT": nc.dram_tensor("xT", [C, S], f32, kind="ExternalInput").ap(),
        "Wqk": nc.dram_tensor("Wqk", [C + 1, 2 * C], f32, kind="ExternalInput").ap(),
        "Wv": nc.dram_tensor("Wv", [C + 1, C], f32, kind="ExternalInput").ap(),
        "Wp": nc.dram_tensor("Wp", [C + 1, C], f32, kind="ExternalInput").ap(),
        "cs": nc.dram_tensor("cs", [128, 2 * S], f32, kind="ExternalInput").ap(),
        "out": nc.dram_tensor("out", [S, C], f32, kind="ExternalOutput").ap(),
    }


def _emit(tc, io=None):
    from concourse import mybir
    from concourse.bass import ds, ts

    nc = tc.nc
    f32 = mybir.dt.float32
    f32r = mybir.dt.float32r
    AF = mybir.ActivationFunctionType
    MUL = mybir.AluOpType.mult
    ADD = mybir.AluOpType.add

    if io is None:
        io = declare_io(nc)
    xT = io["xT"]
    Wqk = io["Wqk"]
    Wv = io["Wv"]
    Wp = io["Wp"]
    cs = io["cs"]
    out = io["out"]

    with ExitStack() as ctx:
        # ---------------- long-lived consts (right side) ----------------
        kons = ctx.enter_context(tc.tile_pool(name="kons", bufs=1, side="right"))
        ones_sb = kons.tile([1, S], f32, name="ones_sb")
        nc.vector.memset(ones_sb[:], 1.0)
        ones_r = kons.tile([1, S], f32r, name="ones_r")
        nc.vector.tensor_copy(ones_r[:], ones_sb[:])
        # loads emitted below (after xk) to keep the startup queues clear
        cs_t = kons.tile([128, 2 * S], f32, name="cs_t")
        bqk2 = kons.tile([128, 16], f32, name="bqk2")

        dstage = ctx.enter_context(tc.tile_pool(name="dstage", bufs=1, space="DRAM"))
        v_dram = dstage.tile([S, H * 65], f32, name="v_dram")
        aT_dram = dstage.tile([C, S], f32, name="aT_dram")

        mm_ps = ctx.enter_context(tc.tile_pool(name="mm_ps", bufs=2, space="PSUM"))

        # ---------------- activations ----------------
        actx = ctx.enter_context(ExitStack())
        xk_p = actx.enter_context(tc.tile_pool(name="xk", bufs=8))
        xk = []
        for k in range(KC):
            t = xk_p.tile([128, S], f32r, name=f"xk{k}", tag="xk")
            xk.append(t)
        for n in range(2):  # halves so the first matmul chain starts early
            for k in range(KC):
                nc.sync.dma_start(
                    out=xk[k][:, ds(n * 512, 512)],
                    in_=xT[ts(k, 128), ds(n * 512, 512)].bitcast(f32r),
                )
        # RoPE tables + qk bias on the SWDGE/Pool queue (idle this early)
        nc.gpsimd.dma_start(out=cs_t[:], in_=cs[:])
        nc.gpsimd.dma_start(
            out=bqk2[:],
            in_=Wqk[C : C + 1, :].rearrange("o (g p) -> (o p) g", p=128),
        )

        wqk_p = actx.enter_context(tc.tile_pool(name="wqk", bufs=3))
        scr_p = actx.enter_context(tc.tile_pool(name="scr", bufs=2))
        tm_p = actx.enter_context(tc.tile_pool(name="tm", bufs=1))
        qkr_p = actx.enter_context(tc.tile_pool(name="qkr", bufs=6))

        # paired W_qk loads: one DMA per pair -> (128, 8k x (2a x 128c))
        wqk_src = Wqk[0:C, :].rearrange(
            "(k p) (a g c) -> p k g a c", p=128, a=2, g=8
        )

        def qk_pair_weights(pair):
            w = wqk_p.tile([128, 8 * 256], f32r, name=f"wqk{pair}", tag="wqk")
            wv4 = w[:].rearrange("p (k a c) -> p k a c", k=8, a=2)
            for a in range(2):
                nc.scalar.dma_start(
                    out=wv4[:, :, a, :],
                    in_=wqk_src[:, :, pair, a, :].bitcast(f32r),
                )
            return w

        def qk_chunk(pair, a, wts):
            """RoPE'd qkT channel chunk gm = a*8 + pair (a=0: q, a=1: k)."""
            gm = a * 8 + pair
            rr = scr_p.tile([128, 2 * S], f32, name=f"rr{gm}", tag="rr")
            for n in range(2):
                ps = mm_ps.tile([128, 512], f32, name=f"qps{gm}_{n}", tag="mm")
                for k in range(KC):
                    nc.tensor.matmul(
                        ps[:],
                        wts[:, k * 256 + a * 128 : k * 256 + a * 128 + 128],
                        xk[k][:, ds(n * 512, 512)],
                        start=(k == 0),
                        stop=(k == KC - 1),
                    )
                # evacuate + per-channel bias (partition dim here) on DVE
                nc.vector.tensor_scalar_add(
                    rr[:, ds(n * 512, 512)], ps[:], bqk2[:, gm : gm + 1]
                )
            # rotate-half copy (partition swap within each 64-row head)
            for d0, s0 in ((0, 32), (32, 0), (64, 96), (96, 64)):
                nc.gpsimd.dma_start(
                    out=rr[d0 : d0 + 32, S : 2 * S], in_=rr[s0 : s0 + 32, 0:S]
                )
            tm = tm_p.tile([128, 2 * S], f32, name=f"tm{gm}", tag="tm")
            nc.vector.tensor_tensor(tm[:], rr[:], cs_t[:], MUL)
            qt = qkr_p.tile([128, S], f32r, name=f"qkr{gm}", tag="qkr")
            nc.vector.tensor_tensor(qt[:], tm[:, 0:S], tm[:, S : 2 * S], ADD)
            return qt

        # -------- pair 0 qk first (early PE work while weights stream) -----
        w0 = qk_pair_weights(0)
        qt0 = qk_chunk(0, 0, w0)
        kt0 = qk_chunk(0, 1, w0)

        # ---------------- v phase ----------------
        with ExitStack() as vctx:
            wv_p = vctx.enter_context(tc.tile_pool(name="wv", bufs=8))
            bias_v = vctx.enter_context(tc.tile_pool(name="bias_v", bufs=1))
            vst_p = vctx.enter_context(tc.tile_pool(name="vst", bufs=3))
            wv = []
            for k in range(KC):
                t = wv_p.tile([128, C], f32r, name=f"wv{k}", tag="wv")
                nc.scalar.dma_start(out=t[:], in_=Wv[ts(k, 128), :].bitcast(f32r))
                wv.append(t)
            bv = bias_v.tile([1, C], f32r, name="bv")
            nc.sync.dma_start(out=bv[:], in_=Wv[C : C + 1, :].bitcast(f32r))

            for mv in range(S // 128):
                vst = vst_p.tile([128, H * 65], f32, name=f"vst{mv}", tag="vst")
                ones_view = vst[:, 0 : H * 65].rearrange("p (h u) -> p h u", u=65)[
                    :, :, 64:65
                ]
                nc.vector.memset(ones_view, 1.0)
                for n in range(2):
                    ps = mm_ps.tile([128, 512], f32, name=f"vps{mv}_{n}", tag="mm")
                    for k in range(KC + 1):
                        if k < KC:
                            lh = xk[k][:, ts(mv, 128)]
                            rh = wv[k][:, ds(n * 512, 512)]
                        else:
                            lh = ones_r[0:1, ts(mv, 128)]
                            rh = bv[0:1, ds(n * 512, 512)]
                        nc.tensor.matmul(
                            ps[:], lh, rh, start=(k == 0), stop=(k == KC)
                        )
                    ov = vst[:, ds(65 * 8 * n, 65 * 8)].rearrange(
                        "p (h u) -> p h u", u=65
                    )[:, :, 0:64]
                    nc.vector.tensor_copy(ov, ps[:])
                nc.gpsimd.dma_start(out=v_dram[ts(mv, 128), :], in_=vst[:])

        # ---------------- attention pools ----------------
        pT_p = actx.enter_context(tc.tile_pool(name="pT", bufs=7))
        vh_p = actx.enter_context(tc.tile_pool(name="vh", bufs=2))
        rec_p = actx.enter_context(tc.tile_pool(name="rec", bufs=2))
        rb_p = actx.enter_context(tc.tile_pool(name="rb", bufs=2))
        tmo_p = actx.enter_context(tc.tile_pool(name="tmo", bufs=3))
        sc_ps = actx.enter_context(tc.tile_pool(name="sc_ps", bufs=2, space="PSUM"))
        oT_ps = actx.enter_context(tc.tile_pool(name="oT_ps", bufs=4, space="PSUM"))

        vh_src = v_dram[0:S, :].rearrange("(k p) c -> p k c", p=128)

        def attn_begin(pair, qtile, ktile):
            """Head-pair attention, even/odd heads interleaved at the sk level
            so their scores matmuls land on disjoint PE row groups (0-63 vs
            64-127) and run concurrently. PV accumulation trails by 2 sk-steps
            to hide the exp (ACT) latency."""
            heads = (2 * pair, 2 * pair + 1)
            vh = vh_p.tile([128, 8 * 130], f32r, name=f"vh{pair}", tag="vh")
            nc.scalar.dma_start(
                out=vh[:].rearrange("p (k c) -> p k c", c=130),
                in_=vh_src[:, :, 130 * pair : 130 * pair + 130].bitcast(f32r),
            )
            Q = {h: qtile[64 * (h % 2) : 64 * (h % 2) + 64, :] for h in heads}
            Kt = {h: ktile[64 * (h % 2) : 64 * (h % 2) + 64, :] for h in heads}
            oT = {
                h: [
                    oT_ps.tile([65, 512], f32, name=f"oT{h}_{n}", tag="oT")
                    for n in range(2)
                ]
                for h in heads
            }
            pT = {}

            def sc_exp(sk):
                for h in heads:
                    pT[(h, sk)] = pT_p.tile(
                        [128, S], f32r, name=f"pT{h}_{sk}", tag="pT"
                    )
                for n in range(2):
                    for h in heads:  # adjacent MMs on disjoint row groups
                        scps = sc_ps.tile(
                            [128, 512], f32, name=f"sc{h}_{sk}_{n}", tag="sc"
                        )
                        nc.tensor.matmul(
                            scps[:],
                            Kt[h][:, ts(sk, 128)],
                            Q[h][:, ds(n * 512, 512)],
                            start=True,
                            stop=True,
                        )
                        nc.scalar.activation(
                            pT[(h, sk)][:, ds(n * 512, 512)],
                            scps[:],
                            AF.Exp,
                            scale=0.125,
                        )

            def pv(sk):
                for n in range(2):
                    for h in heads:
                        c0 = sk * 130 + 65 * (h % 2)
                        nc.tensor.matmul(
                            oT[h][n][:],
                            vh[:, c0 : c0 + 65],
                            pT[(h, sk)][:, ds(n * 512, 512)],
                            start=(sk == 0),
                            stop=(sk == KC - 1),
                        )

            sc_exp(0)
            sc_exp(1)
            for sk in range(2, KC):
                pv(sk - 2)
                sc_exp(sk)
            return heads, oT, pv

        def attn_finish(state):
            heads, oT, pv = state
            pv(KC - 2)
            pv(KC - 1)
            for h in heads:
                rec = rec_p.tile([1, S], f32, name=f"rec{h}", tag="rec")
                for n in range(2):
                    nc.vector.reciprocal(rec[0:1, ds(n * 512, 512)], oT[h][n][64:65, :])
                rb = rb_p.tile([64, S], f32, name=f"rb{h}", tag="rb")
                nc.gpsimd.partition_broadcast(rb[:], rec[:])
                tmo = tmo_p.tile([64, S], f32, name=f"tmo{h}", tag="tmo")
                for n in range(2):
                    nc.vector.tensor_tensor(
                        tmo[:, ds(n * 512, 512)],
                        oT[h][n][0:64, :],
                        rb[:, ds(n * 512, 512)],
                        MUL,
                    )
                nc.gpsimd.dma_start(out=aT_dram[ds(64 * h, 64), :], in_=tmo[:])

        # software pipeline: next pair's qk chunks are emitted inside the
        # window where this pair's last exps are still draining on ACT.
        qt, kt = qt0, kt0
        for pair in range(H // 2):
            state = attn_begin(pair, qt, kt)
            if pair + 1 < H // 2:
                w = qk_pair_weights(pair + 1)
                qt = qk_chunk(pair + 1, 0, w)
                kt = qk_chunk(pair + 1, 1, w)
            attn_finish(state)

        actx.close()

        # ---------------- output projection ----------------
        with ExitStack() as pctx:
            aT_p = pctx.enter_context(tc.tile_pool(name="aT", bufs=8))
            wp_p = pctx.enter_context(tc.tile_pool(name="wp", bufs=8))
            bias_p = pctx.enter_context(tc.tile_pool(name="bias_p", bufs=1))
            ob_p = pctx.enter_context(tc.tile_pool(name="ob", bufs=3))

            aT = []
            wp = []
            for k in range(KC):
                a = aT_p.tile([128, S], f32r, name=f"aT{k}", tag="aT")
                nc.sync.dma_start(out=a[:], in_=aT_dram[ts(k, 128), :].bitcast(f32r))
                aT.append(a)
                w = wp_p.tile([128, C], f32r, name=f"wp{k}", tag="wp")
                nc.scalar.dma_start(out=w[:], in_=Wp[ts(k, 128), :].bitcast(f32r))
                wp.append(w)
            bp = bias_p.tile([1, C], f32r, name="bp")
            nc.sync.dma_start(out=bp[:], in_=Wp[C : C + 1, :].bitcast(f32r))

            for m in range(S // 128):
                ob = ob_p.tile([128, C], f32, name=f"ob{m}", tag="ob")
                for n in range(2):
                    pp = mm_ps.tile([128, 512], f32, name=f"pp{m}_{n}", tag="mm")
                    for k in range(KC + 1):
                        if k < KC:
                            lh = aT[k][:, ts(m, 128)]
                            rh = wp[k][:, ds(n * 512, 512)]
                        else:
                            lh = ones_r[0:1, ts(m, 128)]
                            rh = bp[0:1, ds(n * 512, 512)]
                        nc.tensor.matmul(
                            pp[:], lh, rh, start=(k == 0), stop=(k == KC)
                        )
                    nc.scalar.activation(ob[:, ds(n * 512, 512)], pp[:], AF.Copy)
                nc.sync.dma_start(out=out[ts(m, 128), :], in_=ob[:])


def build_program():
    """Build + compile the Bass program (cached)."""
    if "nc" in _CACHE:
        return _CACHE["nc"]
    import concourse.tile as tile
    from concourse import bacc

    nc = bacc.Bacc(
        "TRN2", target_bir_lowering=False, debug=False, num_devices=N_CORES
    )
    with tile.TileContext(nc) as tc:
        _emit(tc)
    nc.compile()
    _CACHE["nc"] = nc
    return nc


def host_inputs(x, W_qkv, b_qkv, W_proj, b_proj):
    """Per-core input maps (host-side shard + layout prep)."""
    f = np.float32
    x = np.asarray(x, dtype=f)
    W_qkv = np.asarray(W_qkv, dtype=f)
    b_qkv = np.asarray(b_qkv, dtype=f)
    W_proj = np.asarray(W_proj, dtype=f)
    b_proj = np.asarray(b_proj, dtype=f)
    Wqk = np.concatenate([W_qkv[:, : 2 * C], b_qkv[None, : 2 * C]], axis=0)
    Wv = np.concatenate([W_qkv[:, 2 * C :], b_qkv[None, 2 * C :]], axis=0)
    Wp = np.concatenate([W_proj, b_proj[None, :]], axis=0)
    cs = _cs_table()
    maps = []
    for b in range(B):
        maps.append(
            {
                "xT": np.ascontiguousarray(x[b].T),
                "Wqk": np.ascontiguousarray(Wqk),
                "Wv": np.ascontiguousarray(Wv),
                "Wp": np.ascontiguousarray(Wp),
                "cs": cs,
            }
        )
    return maps


def make_runner():
    """Persistent sharded-jit runner (mirrors bass2jax.run_bass_via_pjrt but
    keeps the compiled executable so repeat kernel() calls don't re-compile)."""
    if "runner" in _CACHE:
        return _CACHE["runner"]
    import jax
    from jax.experimental.shard_map import shard_map
    from jax.sharding import Mesh, PartitionSpec
    from concourse import bass2jax, mybir

    nc = build_program()
    bass2jax.install_neuronx_cc_hook()
    partition_name = nc.partition_id_tensor.name if nc.partition_id_tensor else None

    in_names, out_names, out_avals = [], [], []
    for alloc in nc.m.functions[0].allocations:
        if not isinstance(alloc, mybir.MemoryLocationSet):
            continue
        name = alloc.memorylocations[0].name
        if alloc.kind == "ExternalInput":
            if name != partition_name:
                in_names.append(name)
        elif alloc.kind == "ExternalOutput":
            out_names.append(name)
            out_avals.append(
                jax.core.ShapedArray(
                    tuple(alloc.tensor_shape), mybir.dt.np(alloc.dtype)
                )
            )

    all_in_names = in_names + out_names
    if partition_name is not None:
        all_in_names = all_in_names + [partition_name]

    def _body(*args):
        operands = list(args)
        if partition_name is not None:
            operands.append(bass2jax.partition_id_tensor())
        outs = bass2jax._bass_exec_p.bind(
            *operands,
            out_avals=tuple(out_avals),
            in_names=tuple(all_in_names),
            out_names=tuple(out_names),
            lowering_input_output_aliases=(),
            sim_require_finite=True,
            sim_require_nnan=True,
            nc=nc,
        )
        return tuple(outs)

    devices = jax.devices()[:N_CORES]
    mesh = Mesh(np.asarray(devices), ("core",))
    nin = len(in_names) + len(out_names)
    donate = tuple(range(len(in_names), nin))
    sharded = jax.jit(
        shard_map(
            _body,
            mesh=mesh,
            in_specs=(PartitionSpec("core"),) * nin,
            out_specs=(PartitionSpec("core"),) * len(out_names),
            check_rep=False,
        ),
        donate_argnums=donate,
        keep_unused=True,
    )

    def run(in_maps):
        concat_in = [
            np.concatenate([np.asarray(m[name]) for m in in_maps], axis=0)
            for name in in_names
        ]
        zeros = [
            np.zeros((N_CORES * a.shape[0], *a.shape[1:]), a.dtype)
            for a in out_avals
        ]
        outs = sharded(*concat_in, *zeros)
        return {
            name: np.asarray(outs[i]).reshape(N_CORES, *out_avals[i].shape)
            for i, name in enumerate(out_names)
        }

    _CACHE["runner"] = run
    return run


def _install_neff_cache():
    """Memoize the BIR->NEFF compile so repeat kernel() calls skip the
    multi-minute neuronxcc invocation (pure caching, same artifacts)."""
    if _CACHE.get("neff_cache"):
        return
    import hashlib
    import shutil
    import tempfile

    import concourse.bass2jax as b2j
    import concourse.bass_utils as bu

    cache_dir = os.path.join(tempfile.gettempdir(), "bass_neff_cache")
    os.makedirs(cache_dir, exist_ok=True)
    orig = bu.compile_bir_kernel

    def cached(bir_json, tmpdir, neff_name="file.neff"):
        raw = bir_json if isinstance(bir_json, bytes) else bir_json.encode()
        hit = os.path.join(cache_dir, hashlib.sha256(raw).hexdigest() + ".neff")
        if os.path.exists(hit):
            dst = os.path.join(tmpdir, neff_name)
            shutil.copyfile(hit, dst)
            return dst
        path = orig(bir_json, tmpdir, neff_name)
        try:
            shutil.copyfile(path, hit)
        except OSError:
            pass
        return path

    bu.compile_bir_kernel = cached
    b2j.compile_bir_kernel = cached
    _CACHE["neff_cache"] = True


def kernel(x, W_qkv, b_qkv, W_proj, b_proj):
    from concourse.bass_utils import run_bass_kernel_spmd

    _install_neff_cache()
    nc = build_program()
    in_maps = host_inputs(x, W_qkv, b_qkv, W_proj, b_proj)
    res = run_bass_kernel_spmd(nc, in_maps, list(range(N_CORES)))
    return np.stack([r["out"] for r in res.results], axis=0).astype(np.float32)


if __name__ == "__main__":
    nc = build_program()
    print("program built + compiled OK")



# revision 30
# speedup vs baseline: 184.1364x; 184.1364x over previous
"""Trainium2 Bass kernel for nn_Attention_13073880449373.

Full-batch multi-head attention (B=8, S=1024, C=1024, H=16, D=64) with RoPE,
data-parallel over the batch dim: core b computes batch b end-to-end.

v2: all matmul operands in bf16 (the fp32r moving-operand path streams at
~0.5 col/cycle; bf16 streams at full rate), fully SBUF-resident dataflow
(no DRAM staging for V or the attention output), softmax normalization via
reciprocal_approx_fast + gpsimd partition_broadcast per head-pair (replaces
the 1-lane DVE reciprocal chain that stalled the PE every pair), and exp in
one [128,1024] ACT instruction per (head, k-chunk).

Per-core dataflow (all "T" = channels-on-partitions layout):
  xT (C,S) bf16 --[W_qk chunks stationary]--> qkT (2C,S) + bias (DVE evac)
  RoPE on qkT (DVE; 4 small SBUF->SBUF DMAs for the rotate-half swap)
  xT as stationary --> v (S, 16*65) bf16 SBUF tiles, col 64 of each head
                       block = ones (softmax denominator via PV matmul)
  scoresT (Sk,Sq) = kT.T @ qT per head (K=64, even/odd heads on disjoint
                    PE row groups run concurrently)
  pT = exp(0.125*scoresT) (ACT, [128,1024] per instr, bf16 out)
  oT (65,Sq) = [v|1].T @ pT  (row 64 = denominators)
  rec = approx 1/oT[64] (1 custom DVE op), partition_broadcast (GPSIMD),
  normalize (DVE) -> aT bf16 SBUF tiles (proj stationary)
  out (S,C) = aT.T @ W_proj + bias row (K=1 matmul), fp32 out
Pipeline: pair p+1's qk chunks are emitted inside pair p's attention;
pair 0's scores interleave with the v GEMM so ACT never idles there.
"""

import math
import os
from contextlib import ExitStack

import numpy as np

B, S, C = 8, 1024, 1024
H, D = 16, 64
N_CORES = 8
KC = C // 128  # 8 contraction chunks of 128

_CACHE = {}


def _cs_table():
    # Matches reference.rope_cos_sin computed in float32, transposed, with the
    # rotate-half sign folded into the sin half (rows 0-31 negated).
    f = np.float32
    inv = np.exp(np.arange(0, D, 2, dtype=f) * f(-(math.log(10000.0) / D))).astype(f)
    pos = np.arange(S, dtype=f)[:, None]
    ang = (pos * inv[None, :]).astype(f)  # (S, 32)
    ang = np.concatenate([ang, ang], axis=1)  # (S, 64)
    cosT = np.cos(ang).T.astype(f)  # (64, S)
    sinT = np.sin(ang).T.astype(f)
    sign = np.where(np.arange(D) < D // 2, f(-1.0), f(1.0))[:, None].astype(f)
    half = np.concatenate([cosT, sinT * sign], axis=1)  # (64, 2S)
    return np.concatenate([half, half], axis=0).astype(f)  # (128, 2S)


def declare_io(nc):
    from concourse import mybir

    f32 = mybir.dt.float32
    bf16 = mybir.dt.bfloat16
    return {
        "xT": nc.dram_tensor("xT", [C, S], bf16, kind="ExternalInput").ap(),
        "Wqk": nc.dram_tensor("Wqk", [C, 2 * C], bf16, kind="ExternalInput").ap(),
        "bqk": nc.dram_tensor("bqk", [128, 16], f32, kind="ExternalInput").ap(),
        "Wv": nc.dram_tensor("Wv", [C + 1, C], bf16, kind="ExternalInput").ap(),
        "Wp": nc.dram_tensor("Wp", [C + 1, C], bf16, kind="ExternalInput").ap(),
        "cs": nc.dram_tensor("cs", [128, 2 * S], bf16, kind="ExternalInput").ap(),
        "out": nc.dram_tensor("out", [S, C], f32, kind="ExternalOutput").ap(),
    }


def _emit(tc, io=None):
    from concourse import mybir
    from concourse.bass import ds, ts

    nc = tc.nc
    f32 = mybir.dt.float32
    bf16 = mybir.dt.bfloat16
    AF = mybir.ActivationFunctionType
    MUL = mybir.AluOpType.mult
    ADD = mybir.AluOpType.add

    if io is None:
        io = declare_io(nc)
    xT = io["xT"]
    Wqk = io["Wqk"]
    bqk = io["bqk"]
    Wv = io["Wv"]
    Wp = io["Wp"]
    cs = io["cs"]
    out = io["out"]

    with ExitStack() as ctx:
        # ---------------- long-lived consts (right side) ----------------
        kons = ctx.enter_context(tc.tile_pool(name="kons", bufs=1, side="right"))
        ones_sb = kons.tile([1, S], bf16, name="ones_sb")
        nc.vector.memset(ones_sb[:], 1.0)
        cs_t = kons.tile([128, 2 * S], bf16, name="cs_t")
        bqk2 = kons.tile([128, 16], f32, name="bqk2")

        # PSUM: shared stream pool (2x2 banks) + oT pool (2x2 banks) = 8 banks
        ps_p = ctx.enter_context(tc.tile_pool(name="ps", bufs=2, space="PSUM"))
        oT_p = ctx.enter_context(tc.tile_pool(name="oT", bufs=2, space="PSUM"))

        # ---------------- activations ----------------
        xk_p = ctx.enter_context(tc.tile_pool(name="xk", bufs=8))
        xk = []
        for k in range(KC):
            t = xk_p.tile([128, S], bf16, name=f"xk{k}", tag="xk")
            xk.append(t)
        for k in range(KC):  # one full-tile DMA per chunk, first-needed first
            nc.sync.dma_start(out=xk[k][:], in_=xT[ts(k, 128), :])
        # RoPE tables + qk bias on the gpsimd queue (idle this early)
        nc.gpsimd.dma_start(out=cs_t[:], in_=cs[:])
        nc.gpsimd.dma_start(out=bqk2[:], in_=bqk[:])

        wqk_p = ctx.enter_context(tc.tile_pool(name="wqk", bufs=2))
        rr_p = ctx.enter_context(tc.tile_pool(name="rr", bufs=2))
        tm_p = ctx.enter_context(tc.tile_pool(name="tm", bufs=2))
        qkr_p = ctx.enter_context(tc.tile_pool(name="qkr", bufs=6))
        vst_p = ctx.enter_context(tc.tile_pool(name="vst", bufs=8))
        wv_p = ctx.enter_context(tc.tile_pool(name="wv", bufs=8))
        bias_p = ctx.enter_context(tc.tile_pool(name="bias", bufs=1))
        pT_p = ctx.enter_context(tc.tile_pool(name="pT", bufs=16))
        rec_p = ctx.enter_context(tc.tile_pool(name="rec", bufs=2))
        rb_p = ctx.enter_context(tc.tile_pool(name="rb", bufs=2))
        oS_p = ctx.enter_context(tc.tile_pool(name="oS", bufs=2))
        aT_p = ctx.enter_context(tc.tile_pool(name="aT", bufs=8))
        wp_p = ctx.enter_context(tc.tile_pool(name="wp", bufs=8))
        ob_p = ctx.enter_context(tc.tile_pool(name="ob", bufs=2))

        # paired W_qk loads: one DMA per (pair, a) -> (128, 8k x 128c)
        wqk_src = Wqk[0:C, :].rearrange(
            "(k p) (a g c) -> p k g a c", p=128, a=2, g=8
        )

        def qk_pair_weights(pair):
            w = wqk_p.tile([128, 8 * 256], bf16, name=f"wqk{pair}", tag="wqk")
            wv4 = w[:].rearrange("p (k a c) -> p k a c", k=8, a=2)
            for a in range(2):
                nc.scalar.dma_start(
                    out=wv4[:, :, a, :],
                    in_=wqk_src[:, :, pair, a, :],
                )
            return w

        def qk_chunk(pair, a, wts):
            """RoPE'd qkT channel chunk gm = a*8 + pair (a=0: q, a=1: k)."""
            gm = a * 8 + pair
            ps = ps_p.tile([128, S], f32, name=f"qps{gm}", tag="mm")
            for k in range(KC):
                w = wts[:, k * 256 + a * 128 : k * 256 + a * 128 + 128]
                for n in range(2):
                    nc.tensor.matmul(
                        ps[:, ds(n * 512, 512)],
                        w,
                        xk[k][:, ds(n * 512, 512)],
                        start=(k == 0),
                        stop=(k == KC - 1),
                    )
            rr = rr_p.tile([128, 2 * S], bf16, name=f"rr{gm}", tag="rr")
            nc.vector.tensor_scalar_add(rr[:, 0:S], ps[:], bqk2[:, gm : gm + 1])
            # rotate-half copy (partition swap within each 64-row head) on the
            # sync queue: it is idle mid-kernel, and the gpsimd queue would
            # serialize these behind partition_broadcast (which waits on the
            # previous pair's normalize chain).
            for d0, s0 in ((0, 32), (32, 0), (64, 96), (96, 64)):
                nc.sync.dma_start(
                    out=rr[d0 : d0 + 32, S : 2 * S], in_=rr[s0 : s0 + 32, 0:S]
                )
            tm = tm_p.tile([128, 2 * S], bf16, name=f"tm{gm}", tag="tm")
            nc.vector.tensor_tensor(tm[:], rr[:], cs_t[:], MUL)
            qt = qkr_p.tile([128, S], bf16, name=f"qkr{gm}", tag="qkr")
            nc.vector.tensor_tensor(qt[:], tm[:, 0:S], tm[:, S : 2 * S], ADD)
            return qt

        # -------- pair 0 qk first (early PE work while weights stream) -----
        w0 = qk_pair_weights(0)
        qt0 = qk_chunk(0, 0, w0)
        kt0 = qk_chunk(0, 1, w0)

        # ---------------- v phase setup ----------------
        wv = []
        for k in range(KC):
            t = wv_p.tile([128, C], bf16, name=f"wv{k}", tag="wv")
            nc.scalar.dma_start(out=t[:], in_=Wv[ts(k, 128), :])
            wv.append(t)
        bv = bias_p.tile([1, C], bf16, name="bv")
        nc.sync.dma_start(out=bv[:], in_=Wv[C : C + 1, :])
        vst = []

        def v_step(mv):
            t = vst_p.tile([128, H * 65], bf16, name=f"vst{mv}", tag="vst")
            vst.append(t)
            hv = t[:].rearrange("p (h u) -> p h u", u=65)
            nc.vector.memset(hv[:, :, 64:65], 1.0)
            # v streams through the oT-tag PSUM slots: they are idle until
            # pair 0's pv starts, and this keeps the mm slots free for the
            # interleaved pair-0 scores.
            ps = oT_p.tile([128, S], f32, name=f"vps{mv}", tag="oT")
            for k in range(KC + 1):
                if k < KC:
                    lh = xk[k][:, ts(mv, 128)]
                    rh = wv[k]
                else:
                    lh = ones_sb[0:1, ts(mv, 128)]
                    rh = bv
                for n in range(2):
                    nc.tensor.matmul(
                        ps[:, ds(n * 512, 512)],
                        lh,
                        rh[:, ds(n * 512, 512)],
                        start=(k == 0),
                        stop=(k == KC),
                    )
            nc.vector.tensor_copy(hv[:, :, 0:64], ps[:])

        # ---------------- attention helpers ----------------
        def sc_step(pair, sk, qt, kt, pTs):
            """Scores + exp for both heads of the pair at k-chunk sk.
            Even/odd heads sit on PE row groups 0-63 / 64-127, so their
            matmuls run concurrently."""
            heads = (2 * pair, 2 * pair + 1)
            sc = {}
            for h in heads:
                pTs[(h, sk)] = pT_p.tile(
                    [128, S], bf16, name=f"pT{h}_{sk}", tag="pT"
                )
                sc[h] = ps_p.tile([128, S], f32, name=f"sc{h}_{sk}", tag="mm")
            for n in range(2):
                for h in heads:
                    u = h % 2
                    nc.tensor.matmul(
                        sc[h][:, ds(n * 512, 512)],
                        kt[ds(64 * u, 64), ts(sk, 128)],
                        qt[ds(64 * u, 64), ds(n * 512, 512)],
                        start=True,
                        stop=True,
                    )
            for h in heads:
                nc.scalar.activation(pTs[(h, sk)][:], sc[h][:], AF.Exp, scale=0.125)

        def pv_step(pair, sk, oT, pTs):
            heads = (2 * pair, 2 * pair + 1)
            for h in heads:
                for n in range(2):
                    nc.tensor.matmul(
                        oT[h][:, ds(n * 512, 512)],
                        vst[sk][:, 65 * h : 65 * h + 65],
                        pTs[(h, sk)][:, ds(n * 512, 512)],
                        start=(sk == 0),
                        stop=(sk == KC - 1),
                    )

        def attn_finish_a(pair, oT):
            heads = (2 * pair, 2 * pair + 1)
            oS, den = {}, {}
            for h in heads:
                den[h] = rec_p.tile([1, S], f32, name=f"den{h}", tag="den")
                nc.vector.tensor_copy(den[h][0:1, :], oT[h][64:65, :])
                oS[h] = oS_p.tile([64, S], f32, name=f"oS{h}", tag="oS")
                nc.vector.tensor_copy(oS[h][:], oT[h][0:64, :])
            return den, oS

        def attn_finish_b(pair, den, oS):
            aT_t = aT_p.tile([128, S], bf16, name=f"aT{pair}", tag="aT")
            heads = (2 * pair, 2 * pair + 1)
            rb = {}
            for h in heads:
                scr = rec_p.tile([1, S], f32, name=f"scr{h}", tag="scr")
                rec = rec_p.tile([1, S], f32, name=f"rec{h}", tag="rec")
                nc.vector.reciprocal_approx_accurate(
                    out=rec[0:1, :], in_=den[h][0:1, :], scratch=scr[0:1, :]
                )
                rb[h] = rb_p.tile([64, S], f32, name=f"rb{h}", tag="rb")
                nc.gpsimd.partition_broadcast(rb[h][:], rec[0:1, :])
            for h in heads:
                nc.vector.tensor_tensor(
                    aT_t[ds(64 * (h % 2), 64), :], oS[h][:], rb[h][:], MUL
                )
            return aT_t

        def new_oT(pair):
            return {
                h: oT_p.tile([65, S], f32, name=f"oT{h}", tag="oT")
                for h in (2 * pair, 2 * pair + 1)
            }

        aT = []

        # ---------------- pair 0: scores interleaved with the v GEMM -------
        # v first in each slot: the scores wait on the qk RoPE chain, and the
        # PE queue is FIFO, so v matmuls must be ahead of them to fill that
        # window.
        pTs = {}
        w1 = qk_pair_weights(1)
        v_step(0)
        v_step(1)
        for sk in range(KC - 2):
            sc_step(0, sk, qt0, kt0, pTs)
            v_step(sk + 2)
        sc_step(0, KC - 2, qt0, kt0, pTs)
        sc_step(0, KC - 1, qt0, kt0, pTs)
        qt1 = qk_chunk(1, 0, w1)
        kt1 = qk_chunk(1, 1, w1)
        oT = new_oT(0)
        for sk in range(KC):
            pv_step(0, sk, oT, pTs)
        qt, kt = qt1, kt1
        oT_prev, prev_pair = oT, 0

        # ---------------- pairs 1..7 ----------------
        for pair in range(1, H // 2):
            heads_next = pair + 1 < H // 2
            if heads_next:
                wnext = qk_pair_weights(pair + 1)
            if pair == 5:  # proj weights land during late attention
                wp = []
                for k in range(KC):
                    t = wp_p.tile([128, C], bf16, name=f"wp{k}", tag="wp")
                    nc.scalar.dma_start(out=t[:], in_=Wp[ts(k, 128), :])
                    wp.append(t)
                bp = bias_p.tile([1, C], bf16, name="bp")
                nc.sync.dma_start(out=bp[:], in_=Wp[C : C + 1, :])
            pTs = {}
            oT = new_oT(pair)
            sc_step(pair, 0, qt, kt, pTs)
            sc_step(pair, 1, qt, kt, pTs)
            den_, oS_ = attn_finish_a(prev_pair, oT_prev)
            if heads_next:  # early: the RoPE chain needs DVE+DMA slack
                qt_n = qk_chunk(pair + 1, 0, wnext)
            for sk in range(2, KC):
                sc_step(pair, sk, qt, kt, pTs)
                pv_step(pair, sk - 2, oT, pTs)
                if heads_next and sk == 3:
                    kt_n = qk_chunk(pair + 1, 1, wnext)
                if sk == 4:
                    aT.append(attn_finish_b(prev_pair, den_, oS_))
            pv_step(pair, KC - 2, oT, pTs)
            pv_step(pair, KC - 1, oT, pTs)
            oT_prev, prev_pair = oT, pair
            if heads_next:
                qt, kt = qt_n, kt_n
        den_, oS_ = attn_finish_a(prev_pair, oT_prev)
        aT.append(attn_finish_b(prev_pair, den_, oS_))

        # ---------------- output projection ----------------
        # two m-chunks in flight: m+1's k<7 chunks fill the PE while m's k=7
        # waits on the last pair's normalize chain
        def proj_open(m):
            pp = ps_p.tile([128, S], f32, name=f"pp{m}", tag="mm")
            for k in range(KC - 1):
                for n in range(2):
                    nc.tensor.matmul(
                        pp[:, ds(n * 512, 512)],
                        aT[k][:, ts(m, 128)],
                        wp[k][:, ds(n * 512, 512)],
                        start=(k == 0),
                        stop=False,
                    )
            return pp

        def proj_close(m, pp):
            for k in (KC - 1, KC):
                lh = aT[k][:, ts(m, 128)] if k < KC else ones_sb[0:1, ts(m, 128)]
                rh = wp[k] if k < KC else bp
                for n in range(2):
                    nc.tensor.matmul(
                        pp[:, ds(n * 512, 512)],
                        lh,
                        rh[:, ds(n * 512, 512)],
                        start=False,
                        stop=(k == KC),
                    )
            ob = ob_p.tile([128, C], f32, name=f"ob{m}", tag="ob")
            nc.scalar.activation(ob[:], pp[:], AF.Copy)
            nc.sync.dma_start(out=out[ts(m, 128), :], in_=ob[:])

        pp_prev = proj_open(0)
        for m in range(1, S // 128):
            pp_cur = proj_open(m)
            proj_close(m - 1, pp_prev)
            pp_prev = pp_cur
        proj_close(S // 128 - 1, pp_prev)


def build_program():
    """Build + compile the Bass program (cached)."""
    if "nc" in _CACHE:
        return _CACHE["nc"]
    import concourse.tile as tile
    from concourse import bacc

    nc = bacc.Bacc(
        "TRN2", target_bir_lowering=False, debug=False, num_devices=N_CORES
    )
    with tile.TileContext(nc) as tc:
        _emit(tc)
    nc.compile()
    _CACHE["nc"] = nc
    return nc


def host_inputs(x, W_qkv, b_qkv, W_proj, b_proj):
    """Per-core input maps (host-side shard + layout prep)."""
    import ml_dtypes

    f = np.float32
    bf = ml_dtypes.bfloat16
    x = np.asarray(x, dtype=f)
    W_qkv = np.asarray(W_qkv, dtype=f)
    b_qkv = np.asarray(b_qkv, dtype=f)
    W_proj = np.asarray(W_proj, dtype=f)
    b_proj = np.asarray(b_proj, dtype=f)
    Wqk = np.ascontiguousarray(W_qkv[:, : 2 * C]).astype(bf)
    bqk = np.ascontiguousarray(b_qkv[: 2 * C].reshape(16, 128).T).astype(f)
    Wv = np.concatenate(
        [W_qkv[:, 2 * C :], b_qkv[None, 2 * C :]], axis=0
    ).astype(bf)
    Wp = np.concatenate([W_proj, b_proj[None, :]], axis=0).astype(bf)
    cs = _cs_table().astype(bf)
    maps = []
    for b in range(B):
        maps.append(
            {
                "xT": np.ascontiguousarray(x[b].T).astype(bf),
                "Wqk": Wqk,
                "bqk": bqk,
                "Wv": Wv,
                "Wp": Wp,
                "cs": cs,
            }
        )
    return maps


def _install_neff_cache():
    """Memoize the BIR->NEFF compile so repeat kernel() calls skip the
    multi-minute neuronxcc invocation (pure caching, same artifacts)."""
    if _CACHE.get("neff_cache"):
        return
    import hashlib
    import shutil
    import tempfile

    import concourse.bass2jax as b2j
    import concourse.bass_utils as bu

    cache_dir = os.path.join(tempfile.gettempdir(), "bass_neff_cache")
    os.makedirs(cache_dir, exist_ok=True)
    orig = bu.compile_bir_kernel

    def cached(bir_json, tmpdir, neff_name="file.neff"):
        raw = bir_json if isinstance(bir_json, bytes) else bir_json.encode()
        hit = os.path.join(cache_dir, hashlib.sha256(raw).hexdigest() + ".neff")
        if os.path.exists(hit):
            dst = os.path.join(tmpdir, neff_name)
            shutil.copyfile(hit, dst)
            return dst
        path = orig(bir_json, tmpdir, neff_name)
        try:
            shutil.copyfile(path, hit)
        except OSError:
            pass
        return path

    bu.compile_bir_kernel = cached
    b2j.compile_bir_kernel = cached
    _CACHE["neff_cache"] = True


def kernel(x, W_qkv, b_qkv, W_proj, b_proj):
    from concourse.bass_utils import run_bass_kernel_spmd

    _install_neff_cache()
    nc = build_program()
    in_maps = host_inputs(x, W_qkv, b_qkv, W_proj, b_proj)
    res = run_bass_kernel_spmd(nc, in_maps, list(range(N_CORES)))
    return np.stack([r["out"] for r in res.results], axis=0).astype(np.float32)


if __name__ == "__main__":
    nc = build_program()
    print("program built + compiled OK")


# revision 31
# speedup vs baseline: 192.0335x; 1.0429x over previous
"""Trainium2 Bass kernel for nn_Attention_13073880449373.

Full-batch multi-head attention (B=8, S=1024, C=1024, H=16, D=64) with RoPE,
data-parallel over the batch dim: core b computes batch b end-to-end.

v2: all matmul operands in bf16 (the fp32r moving-operand path streams at
~0.5 col/cycle; bf16 streams at full rate), fully SBUF-resident dataflow
(no DRAM staging for V or the attention output), softmax normalization via
reciprocal_approx_fast + gpsimd partition_broadcast per head-pair (replaces
the 1-lane DVE reciprocal chain that stalled the PE every pair), and exp in
one [128,1024] ACT instruction per (head, k-chunk).

Per-core dataflow (all "T" = channels-on-partitions layout):
  xT (C,S) bf16 --[W_qk chunks stationary]--> qkT (2C,S) + bias (DVE evac)
  RoPE on qkT (DVE; 4 small SBUF->SBUF DMAs for the rotate-half swap)
  xT as stationary --> v (S, 16*65) bf16 SBUF tiles, col 64 of each head
                       block = ones (softmax denominator via PV matmul)
  scoresT (Sk,Sq) = kT.T @ qT per head (K=64, even/odd heads on disjoint
                    PE row groups run concurrently)
  pT = exp(0.125*scoresT) (ACT, [128,1024] per instr, bf16 out)
  oT (65,Sq) = [v|1].T @ pT  (row 64 = denominators)
  rec = approx 1/oT[64] (1 custom DVE op), partition_broadcast (GPSIMD),
  normalize (DVE) -> aT bf16 SBUF tiles (proj stationary)
  out (S,C) = aT.T @ W_proj + bias row (K=1 matmul), fp32 out
Pipeline: pair p+1's qk chunks are emitted inside pair p's attention;
pair 0's scores interleave with the v GEMM so ACT never idles there.
"""

import math
import os
from contextlib import ExitStack

import numpy as np

B, S, C = 8, 1024, 1024
H, D = 16, 64
N_CORES = 8
KC = C // 128  # 8 contraction chunks of 128

_CACHE = {}


def _cs_table():
    # Matches reference.rope_cos_sin computed in float32, transposed, with the
    # rotate-half sign folded into the sin half (rows 0-31 negated).
    f = np.float32
    inv = np.exp(np.arange(0, D, 2, dtype=f) * f(-(math.log(10000.0) / D))).astype(f)
    pos = np.arange(S, dtype=f)[:, None]
    ang = (pos * inv[None, :]).astype(f)  # (S, 32)
    ang = np.concatenate([ang, ang], axis=1)  # (S, 64)
    cosT = np.cos(ang).T.astype(f)  # (64, S)
    sinT = np.sin(ang).T.astype(f)
    sign = np.where(np.arange(D) < D // 2, f(-1.0), f(1.0))[:, None].astype(f)
    half = np.concatenate([cosT, sinT * sign], axis=1)  # (64, 2S)
    return np.concatenate([half, half], axis=0).astype(f)  # (128, 2S)


def declare_io(nc):
    from concourse import mybir

    f32 = mybir.dt.float32
    bf16 = mybir.dt.bfloat16
    return {
        "xT": nc.dram_tensor("xT", [C, S], bf16, kind="ExternalInput").ap(),
        "Wqk": nc.dram_tensor("Wqk", [C, 2 * C], bf16, kind="ExternalInput").ap(),
        "bqk": nc.dram_tensor("bqk", [128, 16], f32, kind="ExternalInput").ap(),
        "Wv": nc.dram_tensor("Wv", [C + 1, C], bf16, kind="ExternalInput").ap(),
        "Wp": nc.dram_tensor("Wp", [C + 1, C], bf16, kind="ExternalInput").ap(),
        "cs": nc.dram_tensor("cs", [128, 2 * S], bf16, kind="ExternalInput").ap(),
        "out": nc.dram_tensor("out", [S, C], f32, kind="ExternalOutput").ap(),
    }


def _emit(tc, io=None):
    from concourse import mybir
    from concourse.bass import ds, ts

    nc = tc.nc
    f32 = mybir.dt.float32
    bf16 = mybir.dt.bfloat16
    AF = mybir.ActivationFunctionType
    MUL = mybir.AluOpType.mult
    ADD = mybir.AluOpType.add

    if io is None:
        io = declare_io(nc)
    xT = io["xT"]
    Wqk = io["Wqk"]
    bqk = io["bqk"]
    Wv = io["Wv"]
    Wp = io["Wp"]
    cs = io["cs"]
    out = io["out"]

    with ExitStack() as ctx:
        # ---------------- long-lived consts (right side) ----------------
        kons = ctx.enter_context(tc.tile_pool(name="kons", bufs=1, side="right"))
        ones_sb = kons.tile([1, S], bf16, name="ones_sb")
        nc.vector.memset(ones_sb[:], 1.0)
        cs_t = kons.tile([128, 2 * S], bf16, name="cs_t")
        bqk2 = kons.tile([128, 16], f32, name="bqk2")

        # PSUM: shared stream pool (2x2 banks) + oT pool (2x2 banks) = 8 banks
        ps_p = ctx.enter_context(tc.tile_pool(name="ps", bufs=2, space="PSUM"))
        oT_p = ctx.enter_context(tc.tile_pool(name="oT", bufs=2, space="PSUM"))

        # ---------------- activations ----------------
        xk_p = ctx.enter_context(tc.tile_pool(name="xk", bufs=8))
        xk = []
        for k in range(KC):
            t = xk_p.tile([128, S], bf16, name=f"xk{k}", tag="xk")
            xk.append(t)
        for k in range(KC):  # one full-tile DMA per chunk, first-needed first
            nc.sync.dma_start(out=xk[k][:], in_=xT[ts(k, 128), :])
        # RoPE tables + qk bias on the gpsimd queue (idle this early)
        nc.gpsimd.dma_start(out=cs_t[:], in_=cs[:])
        nc.gpsimd.dma_start(out=bqk2[:], in_=bqk[:])

        wqk_p = ctx.enter_context(tc.tile_pool(name="wqk", bufs=2))
        rr_p = ctx.enter_context(tc.tile_pool(name="rr", bufs=2))
        tm_p = ctx.enter_context(tc.tile_pool(name="tm", bufs=2))
        qkr_p = ctx.enter_context(tc.tile_pool(name="qkr", bufs=6))
        vst_p = ctx.enter_context(tc.tile_pool(name="vst", bufs=8))
        wv_p = ctx.enter_context(tc.tile_pool(name="wv", bufs=8))
        bias_p = ctx.enter_context(tc.tile_pool(name="bias", bufs=1))
        pT_p = ctx.enter_context(tc.tile_pool(name="pT", bufs=16))
        rec_p = ctx.enter_context(tc.tile_pool(name="rec", bufs=2))
        rb_p = ctx.enter_context(tc.tile_pool(name="rb", bufs=2))
        oS_p = ctx.enter_context(tc.tile_pool(name="oS", bufs=2))
        aT_p = ctx.enter_context(tc.tile_pool(name="aT", bufs=8))
        wp_p = ctx.enter_context(tc.tile_pool(name="wp", bufs=8))
        ob_p = ctx.enter_context(tc.tile_pool(name="ob", bufs=2))

        # W_qk is pre-arranged host-side to (pair, p, k*256+a*128+c): each
        # pair's weights are one contiguous [128, 2048] DMA (the strided
        # gather form took ~3x longer to land and gated the first matmul)
        def qk_pair_weights(pair):
            w = wqk_p.tile([128, 8 * 256], bf16, name=f"wqk{pair}", tag="wqk")
            nc.scalar.dma_start(out=w[:], in_=Wqk[ts(pair, 128), :])
            return w

        def qk_chunk(pair, a, wts):
            """RoPE'd qkT channel chunk gm = a*8 + pair (a=0: q, a=1: k)."""
            gm = a * 8 + pair
            ps = ps_p.tile([128, S], f32, name=f"qps{gm}", tag="mm")
            for k in range(KC):
                w = wts[:, k * 256 + a * 128 : k * 256 + a * 128 + 128]
                for n in range(2):
                    nc.tensor.matmul(
                        ps[:, ds(n * 512, 512)],
                        w,
                        xk[k][:, ds(n * 512, 512)],
                        start=(k == 0),
                        stop=(k == KC - 1),
                    )
            rr = rr_p.tile([128, 2 * S], bf16, name=f"rr{gm}", tag="rr")
            nc.vector.tensor_scalar_add(rr[:, 0:S], ps[:], bqk2[:, gm : gm + 1])
            # rotate-half copy (partition swap within each 64-row head) on the
            # sync queue: it is idle mid-kernel, and the gpsimd queue would
            # serialize these behind partition_broadcast (which waits on the
            # previous pair's normalize chain).
            for d0, s0 in ((0, 32), (32, 0), (64, 96), (96, 64)):
                nc.sync.dma_start(
                    out=rr[d0 : d0 + 32, S : 2 * S], in_=rr[s0 : s0 + 32, 0:S]
                )
            tm = tm_p.tile([128, 2 * S], bf16, name=f"tm{gm}", tag="tm")
            nc.vector.tensor_tensor(tm[:], rr[:], cs_t[:], MUL)
            qt = qkr_p.tile([128, S], bf16, name=f"qkr{gm}", tag="qkr")
            nc.vector.tensor_tensor(qt[:], tm[:, 0:S], tm[:, S : 2 * S], ADD)
            return qt

        # -------- pair 0 qk first (early PE work while weights stream) -----
        w0 = qk_pair_weights(0)
        qt0 = qk_chunk(0, 0, w0)
        kt0 = qk_chunk(0, 1, w0)

        # ---------------- v phase setup ----------------
        wv = []
        for k in range(KC):
            t = wv_p.tile([128, C], bf16, name=f"wv{k}", tag="wv")
            nc.scalar.dma_start(out=t[:], in_=Wv[ts(k, 128), :])
            wv.append(t)
        bv = bias_p.tile([1, C], bf16, name="bv")
        nc.sync.dma_start(out=bv[:], in_=Wv[C : C + 1, :])
        vst = []

        def v_step(mv):
            t = vst_p.tile([128, H * 65], bf16, name=f"vst{mv}", tag="vst")
            vst.append(t)
            hv = t[:].rearrange("p (h u) -> p h u", u=65)
            nc.vector.memset(hv[:, :, 64:65], 1.0)
            # v streams through the oT-tag PSUM slots: they are idle until
            # pair 0's pv starts, and this keeps the mm slots free for the
            # interleaved pair-0 scores.
            ps = oT_p.tile([128, S], f32, name=f"vps{mv}", tag="oT")
            for k in range(KC + 1):
                if k < KC:
                    lh = xk[k][:, ts(mv, 128)]
                    rh = wv[k]
                else:
                    lh = ones_sb[0:1, ts(mv, 128)]
                    rh = bv
                for n in range(2):
                    nc.tensor.matmul(
                        ps[:, ds(n * 512, 512)],
                        lh,
                        rh[:, ds(n * 512, 512)],
                        start=(k == 0),
                        stop=(k == KC),
                    )
            nc.vector.tensor_copy(hv[:, :, 0:64], ps[:])

        # ---------------- attention helpers ----------------
        def sc_step(pair, sk, qt, kt, pTs):
            """Scores + exp for both heads of the pair at k-chunk sk.
            Even/odd heads sit on PE row groups 0-63 / 64-127, so their
            matmuls run concurrently."""
            heads = (2 * pair, 2 * pair + 1)
            sc = {}
            for h in heads:
                pTs[(h, sk)] = pT_p.tile(
                    [128, S], bf16, name=f"pT{h}_{sk}", tag="pT"
                )
                sc[h] = ps_p.tile([128, S], f32, name=f"sc{h}_{sk}", tag="mm")
            for n in range(2):
                for h in heads:
                    u = h % 2
                    nc.tensor.matmul(
                        sc[h][:, ds(n * 512, 512)],
                        kt[ds(64 * u, 64), ts(sk, 128)],
                        qt[ds(64 * u, 64), ds(n * 512, 512)],
                        start=True,
                        stop=True,
                    )
            for h in heads:
                nc.scalar.activation(pTs[(h, sk)][:], sc[h][:], AF.Exp, scale=0.125)

        def pv_step(pair, sk, oT, pTs):
            heads = (2 * pair, 2 * pair + 1)
            for h in heads:
                for n in range(2):
                    nc.tensor.matmul(
                        oT[h][:, ds(n * 512, 512)],
                        vst[sk][:, 65 * h : 65 * h + 65],
                        pTs[(h, sk)][:, ds(n * 512, 512)],
                        start=(sk == 0),
                        stop=(sk == KC - 1),
                    )

        def attn_finish_a(pair, oT):
            heads = (2 * pair, 2 * pair + 1)
            oS, den = {}, {}
            for h in heads:
                den[h] = rec_p.tile([1, S], f32, name=f"den{h}", tag="den")
                nc.vector.tensor_copy(den[h][0:1, :], oT[h][64:65, :])
                oS[h] = oS_p.tile([64, S], f32, name=f"oS{h}", tag="oS")
                nc.vector.tensor_copy(oS[h][:], oT[h][0:64, :])
            return den, oS

        def attn_finish_b(pair, den, oS):
            aT_t = aT_p.tile([128, S], bf16, name=f"aT{pair}", tag="aT")
            heads = (2 * pair, 2 * pair + 1)
            rb = {}
            for h in heads:
                scr = rec_p.tile([1, S], f32, name=f"scr{h}", tag="scr")
                rec = rec_p.tile([1, S], f32, name=f"rec{h}", tag="rec")
                nc.vector.reciprocal_approx_accurate(
                    out=rec[0:1, :], in_=den[h][0:1, :], scratch=scr[0:1, :]
                )
                rb[h] = rb_p.tile([64, S], f32, name=f"rb{h}", tag="rb")
                nc.gpsimd.partition_broadcast(rb[h][:], rec[0:1, :])
            for h in heads:
                nc.vector.tensor_tensor(
                    aT_t[ds(64 * (h % 2), 64), :], oS[h][:], rb[h][:], MUL
                )
            return aT_t

        def new_oT(pair):
            return {
                h: oT_p.tile([65, S], f32, name=f"oT{h}", tag="oT")
                for h in (2 * pair, 2 * pair + 1)
            }

        aT = []

        # ---------------- pair 0: scores interleaved with the v GEMM -------
        # v first in each slot: the scores wait on the qk RoPE chain, and the
        # PE queue is FIFO, so v matmuls must be ahead of them to fill that
        # window.
        pTs = {}
        w1 = qk_pair_weights(1)
        v_step(0)
        v_step(1)
        for sk in range(KC - 2):
            sc_step(0, sk, qt0, kt0, pTs)
            v_step(sk + 2)
            if sk == 2:
                qt1 = qk_chunk(1, 0, w1)
            if sk == 4:
                kt1 = qk_chunk(1, 1, w1)
        sc_step(0, KC - 2, qt0, kt0, pTs)
        sc_step(0, KC - 1, qt0, kt0, pTs)
        oT = new_oT(0)
        for sk in range(KC):
            pv_step(0, sk, oT, pTs)
        qt, kt = qt1, kt1
        oT_prev, prev_pair = oT, 0

        # ---------------- pairs 1..7 ----------------
        for pair in range(1, H // 2):
            heads_next = pair + 1 < H // 2
            if heads_next:
                wnext = qk_pair_weights(pair + 1)
            if pair == 5:  # proj weights land during late attention
                wp = []
                for k in range(KC):
                    t = wp_p.tile([128, C], bf16, name=f"wp{k}", tag="wp")
                    nc.scalar.dma_start(out=t[:], in_=Wp[ts(k, 128), :])
                    wp.append(t)
                bp = bias_p.tile([1, C], bf16, name="bp")
                nc.sync.dma_start(out=bp[:], in_=Wp[C : C + 1, :])
            pTs = {}
            oT = new_oT(pair)
            sc_step(pair, 0, qt, kt, pTs)
            sc_step(pair, 1, qt, kt, pTs)
            den_, oS_ = attn_finish_a(prev_pair, oT_prev)
            if heads_next:  # early: the RoPE chain needs DVE+DMA slack
                qt_n = qk_chunk(pair + 1, 0, wnext)
            for sk in range(2, KC):
                sc_step(pair, sk, qt, kt, pTs)
                pv_step(pair, sk - 2, oT, pTs)
                if heads_next and sk == 3:
                    kt_n = qk_chunk(pair + 1, 1, wnext)
                if sk == 4:
                    aT.append(attn_finish_b(prev_pair, den_, oS_))
            pv_step(pair, KC - 2, oT, pTs)
            pv_step(pair, KC - 1, oT, pTs)
            oT_prev, prev_pair = oT, pair
            if heads_next:
                qt, kt = qt_n, kt_n
        # last pair: no successor needs the PSUM slots, so skip the oS
        # evacuation and multiply straight out of PSUM (shortens the chain
        # that gates the projection's k=7 accumulation)
        aT_t = aT_p.tile([128, S], bf16, name="aT7", tag="aT")
        for h in (2 * prev_pair, 2 * prev_pair + 1):
            u = h % 2
            den = rec_p.tile([1, S], f32, name=f"denL{h}", tag="den")
            nc.vector.tensor_copy(den[0:1, :], oT_prev[h][64:65, :])
            scr = rec_p.tile([1, S], f32, name=f"scrL{h}", tag="scr")
            rec = rec_p.tile([1, S], f32, name=f"recL{h}", tag="rec")
            nc.vector.reciprocal_approx_accurate(
                out=rec[0:1, :], in_=den[0:1, :], scratch=scr[0:1, :]
            )
            rb = rb_p.tile([64, S], f32, name=f"rbL{h}", tag="rb")
            nc.gpsimd.partition_broadcast(rb[:], rec[0:1, :])
            nc.vector.tensor_tensor(
                aT_t[ds(64 * u, 64), :], oT_prev[h][0:64, :], rb[:], MUL
            )
        aT.append(aT_t)

        # ---------------- output projection ----------------
        # two m-chunks in flight: m+1's k<7 chunks fill the PE while m's k=7
        # waits on the last pair's normalize chain
        def proj_open(m):
            pp = ps_p.tile([128, S], f32, name=f"pp{m}", tag="mm")
            for k in range(KC - 1):
                for n in range(2):
                    nc.tensor.matmul(
                        pp[:, ds(n * 512, 512)],
                        aT[k][:, ts(m, 128)],
                        wp[k][:, ds(n * 512, 512)],
                        start=(k == 0),
                        stop=False,
                    )
            return pp

        def proj_close(m, pp):
            for k in (KC - 1, KC):
                lh = aT[k][:, ts(m, 128)] if k < KC else ones_sb[0:1, ts(m, 128)]
                rh = wp[k] if k < KC else bp
                for n in range(2):
                    nc.tensor.matmul(
                        pp[:, ds(n * 512, 512)],
                        lh,
                        rh[:, ds(n * 512, 512)],
                        start=False,
                        stop=(k == KC),
                    )
            ob = ob_p.tile([128, C], f32, name=f"ob{m}", tag="ob")
            nc.scalar.activation(ob[:], pp[:], AF.Copy)
            nc.sync.dma_start(out=out[ts(m, 128), :], in_=ob[:])

        pp_prev = proj_open(0)
        for m in range(1, S // 128):
            pp_cur = proj_open(m)
            proj_close(m - 1, pp_prev)
            pp_prev = pp_cur
        proj_close(S // 128 - 1, pp_prev)


def build_program():
    """Build + compile the Bass program (cached)."""
    if "nc" in _CACHE:
        return _CACHE["nc"]
    import concourse.tile as tile
    from concourse import bacc

    nc = bacc.Bacc(
        "TRN2", target_bir_lowering=False, debug=False, num_devices=N_CORES
    )
    with tile.TileContext(nc) as tc:
        _emit(tc)
    nc.compile()
    _CACHE["nc"] = nc
    return nc


def host_inputs(x, W_qkv, b_qkv, W_proj, b_proj):
    """Per-core input maps (host-side shard + layout prep)."""
    import ml_dtypes

    f = np.float32
    bf = ml_dtypes.bfloat16
    x = np.asarray(x, dtype=f)
    W_qkv = np.asarray(W_qkv, dtype=f)
    b_qkv = np.asarray(b_qkv, dtype=f)
    W_proj = np.asarray(W_proj, dtype=f)
    b_proj = np.asarray(b_proj, dtype=f)
    Wq4 = W_qkv[:, : 2 * C].reshape(8, 128, 2, 8, 128)  # (k, p, a, g, c)
    Wqk = np.ascontiguousarray(
        Wq4.transpose(3, 1, 0, 2, 4).reshape(8 * 128, 2 * C)
    ).astype(bf)  # (g*128+p, k*256+a*128+c)
    bqk = np.ascontiguousarray(b_qkv[: 2 * C].reshape(16, 128).T).astype(f)
    Wv = np.concatenate(
        [W_qkv[:, 2 * C :], b_qkv[None, 2 * C :]], axis=0
    ).astype(bf)
    Wp = np.concatenate([W_proj, b_proj[None, :]], axis=0).astype(bf)
    cs = _cs_table().astype(bf)
    maps = []
    for b in range(B):
        maps.append(
            {
                "xT": np.ascontiguousarray(x[b].T).astype(bf),
                "Wqk": Wqk,
                "bqk": bqk,
                "Wv": Wv,
                "Wp": Wp,
                "cs": cs,
            }
        )
    return maps


def _install_neff_cache():
    """Memoize the BIR->NEFF compile so repeat kernel() calls skip the
    multi-minute neuronxcc invocation (pure caching, same artifacts)."""
    if _CACHE.get("neff_cache"):
        return
    import hashlib
    import shutil
    import tempfile

    import concourse.bass2jax as b2j
    import concourse.bass_utils as bu

    cache_dir = os.path.join(tempfile.gettempdir(), "bass_neff_cache")
    os.makedirs(cache_dir, exist_ok=True)
    orig = bu.compile_bir_kernel

    def cached(bir_json, tmpdir, neff_name="file.neff"):
        raw = bir_json if isinstance(bir_json, bytes) else bir_json.encode()
        hit = os.path.join(cache_dir, hashlib.sha256(raw).hexdigest() + ".neff")
        if os.path.exists(hit):
            dst = os.path.join(tmpdir, neff_name)
            shutil.copyfile(hit, dst)
            return dst
        path = orig(bir_json, tmpdir, neff_name)
        try:
            shutil.copyfile(path, hit)
        except OSError:
            pass
        return path

    bu.compile_bir_kernel = cached
    b2j.compile_bir_kernel = cached
    _CACHE["neff_cache"] = True


def kernel(x, W_qkv, b_qkv, W_proj, b_proj):
    from concourse.bass_utils import run_bass_kernel_spmd

    _install_neff_cache()
    nc = build_program()
    in_maps = host_inputs(x, W_qkv, b_qkv, W_proj, b_proj)
    res = run_bass_kernel_spmd(nc, in_maps, list(range(N_CORES)))
    return np.stack([r["out"] for r in res.results], axis=0).astype(np.float32)


if __name__ == "__main__":
    nc = build_program()
    print("program built + compiled OK")
